# revision 1
# baseline (speedup 1.0000x reference)
"""GAT (dense adjacency, 4-head, concat) + BatchNorm + ReLU on 8 TRN2 cores.

Math: h = x@W per head; scores s[n,m] = ei[n]+ej[m] (rank-1!);
att = softmax_m(mask(leaky(s))). Since exp(leaky(s))/exp(ei[n]) =
max(q1[m], q2[m]*w[n]) with q1=exp(ej), q2=exp(0.2*ej), w=exp(-0.8*ei),
and the exp(ei[n]) factor cancels in the softmax normalization, each
device only does 2 elementwise passes over its [8192, 1024] score block
(computed transposed so PE aggregates without any transposes; a ones
column in the rhs yields the softmax denominator for free).

Sharding: rows (target nodes) split across 8 cores; every core computes
h for all nodes from x (cheap) instead of an all-gather.
"""

import sys

sys.path.insert(0, "/opt/trn_rl_repo")

import numpy as np
import ml_dtypes

import concourse.bass as bass
import concourse.mybir as mybir
from concourse import tile
from concourse.bass_utils import run_bass_kernel_spmd

F32 = mybir.dt.float32
BF16 = mybir.dt.bfloat16
AF = mybir.ActivationFunctionType
OP = mybir.AluOpType

N, IN, OUT, H = 8192, 128, 64, 4
NCORES = 8
EPS = 1e-5

# Fraction of (m-tile, head) units on "Path C" (ACT relu + 1 DVE op) vs
# "Path A" (2 DVE ops); balances the VectorE and ScalarE engines.
PATH_C_NUM, PATH_C_DEN = 7, 10


def legalize_waits(nc, max_waits=1):
    """Walrus in this container encodes at most one inline sem-wait per
    engine instruction; hoist extras onto single-wait NoOps placed before."""
    nid = 0
    for f in nc.m.functions:
        for bb in f.blocks:
            new = []
            for inst in bb.instructions:
                si = inst.sync_info
                if si is not None and si.on_wait and len(si.on_wait) > max_waits:
                    waits = list(si.on_wait)
                    head, tail = waits[:-max_waits], waits[-max_waits:]
                    for w in head:
                        nid += 1
                        new.append(mybir.InstNoOp(
                            name=f"LGW-{nid}", ins=[], outs=[],
                            engine=inst.engine,
                            sync_info=mybir.SyncInfo(on_wait=[w], on_update=[]),
                            bass_nofuse=True,
                        ))
                    inst.sync_info = mybir.SyncInfo(
                        on_wait=tail, on_update=list(si.on_update)
                    )
                new.append(inst)
            bb.instructions = new
    return nc


def build_kernel(n_nodes=N, n_cores=NCORES, reps=1):
    """Build the per-core Bass program (SPMD: same program, per-core inputs)."""
    nblk = n_nodes // n_cores          # rows (target nodes) per core
    n_mt = n_nodes // 128              # m-tiles (source-node tiles of 128)
    jc = min(512, nblk)                # column-chunk width for PE moving dim
    n_jt = nblk // jc                  # column chunks of the n block
    WAUG = H * 66                      # per head: 64 h cols + ei + ej

    # all small inputs packed into one tensor -> ONE DMA on ONE HW queue, so
    # the first PE matmul carries a single semaphore wait (walrus limit)
    CW = n_nodes + nblk + WAUG + H + 128
    nc = bass.Bass()
    consts_d = nc.dram_tensor("consts", [IN, CW], F32, kind="ExternalInput")
    adjT_d = nc.dram_tensor("adjT", [n_nodes, nblk], BF16, kind="ExternalInput")
    out_d = nc.dram_tensor("out", [H, OUT + 1, nblk], F32, kind="ExternalOutput")

    with tile.TileContext(nc) as tc:
      for _rep in range(reps):
        with (
            tc.tile_pool(name="const", bufs=1) as cpool,
            tc.tile_pool(name="persist", bufs=1) as ppool,
            tc.tile_pool(name="stream", bufs=3) as spool,
            tc.tile_pool(name="score", bufs=4) as epool,
        ):
            # ---- load constants (one DMA) ----
            consts = cpool.tile([IN, CW], F32, tag="consts")
            nc.sync.dma_start(consts[:], consts_d[:])
            xT = consts[:, 0:n_nodes]
            xTown = consts[:, n_nodes:n_nodes + nblk]
            waug = consts[:, n_nodes + nblk:n_nodes + nblk + WAUG]
            wa = consts[:, n_nodes + nblk + WAUG:n_nodes + nblk + WAUG + H]
            ones_row = consts[0:1, CW - 128:CW]   # [1,128] of 1.0

            # ---- persistent per-head state ----
            h_aug, q1, q2, nq1, w_bc = [], [], [], [], []
            for hd in range(H):
                h_aug.append(ppool.tile([128, n_mt * 65], BF16, tag=f"haug{hd}", name=f"haug{hd}"))
                q1.append(ppool.tile([128, n_mt], F32, tag=f"q1_{hd}", name=f"q1_{hd}"))
                q2.append(ppool.tile([128, n_mt], F32, tag=f"q2_{hd}", name=f"q2_{hd}"))
                nq1.append(ppool.tile([128, n_mt], F32, tag=f"nq1_{hd}", name=f"nq1_{hd}"))
                w_bc.append(ppool.tile([128, nblk], BF16, tag=f"wbc{hd}", name=f"wbc{hd}"))
                # ones column of h_aug (col 64 of each 65-block) survives the
                # h copies below
                nc.gpsimd.memset(h_aug[hd][:], 1.0)

            # ---- phase A: h, ei, ej for all nodes; w for own rows ----
            # ei/ej fused into the h matmul: Waug cols per head = [W | W@a_i | W@a_j]
            pha = tc.tile_pool(name="psA", bufs=2, space="PSUM")
            psA_pool = pha.__enter__()
            phw = tc.tile_pool(name="psW", bufs=1, space="PSUM")
            psW_pool = phw.__enter__()
            for t in range(n_mt):
                psA = psA_pool.tile([128, WAUG], F32, tag="psA")
                nc.tensor.matmul(
                    psA[:], xT[:, t * 128:(t + 1) * 128], waug[:],
                    start=True, stop=True,
                )
                for hd in range(H):
                    c0 = hd * 66
                    nc.scalar.activation(
                        h_aug[hd][:, t * 65:t * 65 + 64], psA[:, c0:c0 + 64], AF.Copy
                    )
                    nc.scalar.activation(
                        q1[hd][:, t:t + 1], psA[:, c0 + 65:c0 + 66], AF.Exp
                    )
                    nc.scalar.activation(
                        q2[hd][:, t:t + 1], psA[:, c0 + 65:c0 + 66], AF.Exp, scale=0.2
                    )
            for hd in range(H):
                nc.vector.tensor_scalar_mul(nq1[hd][:], q1[hd][:], -1.0)
                # w[n] = exp(-0.8 * ei[n]) for own rows, in free-dim layout:
                # ei_row = wa_i[hd] @ xTown via PE (lhsT free dim = 1)
                eiT = psA_pool.tile([1, nblk], F32, tag="eiT")
                for j in range(n_jt):
                    nc.tensor.matmul(
                        eiT[:, j * jc:(j + 1) * jc],
                        wa[:, hd:hd + 1], xTown[:, j * jc:(j + 1) * jc],
                        start=True, stop=True,
                    )
                # broadcast ei row to all partitions via PE (ones ⊗ row),
                # then w = exp(-0.8*ei) on the PSUM->SBUF copy
                ei_row = spool.tile([1, nblk], F32, tag="eirow")
                nc.scalar.activation(ei_row[:], eiT[:], AF.Copy)
                psW = psW_pool.tile([128, nblk], F32, tag="psW")
                for j in range(n_jt):
                    nc.tensor.matmul(
                        psW[:, j * jc:(j + 1) * jc],
                        ones_row[:, :], ei_row[0:1, j * jc:(j + 1) * jc],
                        start=True, stop=True,
                    )
                nc.scalar.activation(w_bc[hd][:], psW[:], AF.Exp, scale=-0.8)

            phw.__exit__(None, None, None)
            pha.__exit__(None, None, None)
            # all-engine barrier: afterwards every engine's vector clock has
            # observed phase A, so each phase-B matmul needs <=1 sem wait
            tc.strict_bb_all_engine_barrier()

            # ---- phase B: masked attention + aggregation over m-tiles ----
            phb = tc.tile_pool(name="psB", bufs=1, space="PSUM")
            psB_pool = phb.__enter__()
            ps_out = [
                psB_pool.tile([OUT + 1, nblk], F32, tag=f"psB{hd}", name=f"psB{hd}") for hd in range(H)
            ]
            unit = 0
            for t in range(n_mt):
                adjt = spool.tile([128, nblk], BF16, tag="adjt")
                nc.sync.dma_start(adjt[:], adjT_d[t * 128:(t + 1) * 128, :])
                for hd in range(H):
                    q1s = q1[hd][:, t:t + 1]
                    q2s = q2[hd][:, t:t + 1]
                    E = epool.tile([128, nblk], BF16, tag="E")
                    if unit % PATH_C_DEN < PATH_C_NUM:
                        # Path C: r = relu(q2*w - q1) on ACT; E = (r+q1)*adj on DVE
                        r = epool.tile([128, nblk], BF16, tag="r")
                        nc.scalar.activation(
                            r[:], w_bc[hd][:], AF.Relu,
                            bias=nq1[hd][:, t:t + 1], scale=q2s,
                        )
                        nc.vector.scalar_tensor_tensor(
                            E[:], r[:], q1s, adjt[:], OP.add, OP.mult
                        )
                    else:
                        # Path A: a = max(q2*w, q1); E = a*adj (both on DVE)
                        a = epool.tile([128, nblk], BF16, tag="r")
                        nc.vector.tensor_scalar(
                            a[:], w_bc[hd][:], q2s, q1s, OP.mult, OP.max
                        )
                        nc.vector.tensor_tensor(E[:], a[:], adjt[:], OP.mult)
                    unit += 1
                    for j in range(n_jt):
                        nc.tensor.matmul(
                            ps_out[hd][:, j * jc:(j + 1) * jc],
                            h_aug[hd][:, t * 65:(t + 1) * 65],
                            E[:, j * jc:(j + 1) * jc],
                            start=(t == 0), stop=(t == n_mt - 1),
                        )

            # ---- phase C: emit [h + rowsum] rows; normalization on host ----
            for hd in range(H):
                o = spool.tile([OUT + 1, nblk], F32, tag="onorm")
                nc.scalar.activation(o[:], ps_out[hd][:], AF.Copy)
                nc.sync.dma_start(out_d[hd], o[:])
            phb.__exit__(None, None, None)

    return nc


_CACHE = {}


def _get_nc(n_nodes, n_cores):
    key = (n_nodes, n_cores)
    if key not in _CACHE:
        _CACHE[key] = legalize_waits(build_kernel(n_nodes, n_cores))
    return _CACHE[key]


def make_in_maps(x, adj, W, a_i, a_j, n_cores=NCORES):
    n_nodes = x.shape[0]
    nblk = n_nodes // n_cores
    xT = np.ascontiguousarray(x.T).astype(np.float32)
    adjT = np.ascontiguousarray(adj.T).astype(ml_dtypes.bfloat16)
    WAUGW = H * 66
    waug = np.zeros((IN, H, 66), np.float32)
    wa = np.zeros((IN, H), np.float32)
    for hd in range(H):
        waug[:, hd, 0:64] = W[hd]
        waug[:, hd, 64] = W[hd] @ a_i[hd]
        waug[:, hd, 65] = W[hd] @ a_j[hd]
        wa[:, hd] = W[hd] @ a_i[hd]
    waug = waug.reshape(IN, WAUGW)
    maps = []
    for c in range(n_cores):
        sl = slice(c * nblk, (c + 1) * nblk)
        ones = np.zeros((IN, 128), np.float32)
        ones[0, :] = 1.0
        consts = np.concatenate(
            [xT, xT[:, sl], waug, wa, ones], axis=1
        ).astype(np.float32)
        maps.append({
            "consts": np.ascontiguousarray(consts),
            "adjT": np.ascontiguousarray(adjT[:, sl]),
        })
    return maps


def postprocess(results, gamma, beta, n_cores=NCORES):
    """Per-core [H, 65, nblk] -> full [N, H*OUT] with softmax-norm + BN + ReLU."""
    blocks = []
    for c in range(n_cores):
        r = results[c]["out"]                      # [H, 65, nblk]
        o = r[:, :OUT, :] / r[:, OUT:OUT + 1, :]   # softmax normalize
        # [H, OUT, nblk] -> [nblk, H*OUT]
        blocks.append(np.transpose(o, (2, 0, 1)).reshape(-1, H * OUT))
    out = np.concatenate(blocks, axis=0).astype(np.float32)
    mean = out.mean(axis=0)
    var = out.var(axis=0)
    out = (out - mean) * (1.0 / np.sqrt(var + EPS)) * gamma + beta
    return np.maximum(out, 0.0).astype(np.float32)


def kernel(x, adj, W, a_i, a_j, gamma, beta):
    nc = _get_nc(N, NCORES)
    in_maps = make_in_maps(x, adj, W, a_i, a_j, NCORES)
    res = run_bass_kernel_spmd(nc, in_maps, list(range(NCORES)))
    return postprocess(res.results, np.asarray(gamma), np.asarray(beta), NCORES)



# revision 21
# speedup vs baseline: 25.8103x; 25.8103x over previous
"""GAT (dense adjacency, 4-head, concat) + BatchNorm + ReLU on 8 TRN2 cores.

Math: h = x@W per head; scores s[n,m] = ei[n]+ej[m] (rank-1!);
att = softmax_m(mask(leaky(s))). Since exp(leaky(s))/exp(ei[n]) =
max(q1[m], q2[m]*w[n]) with q1=exp(ej), q2=exp(0.2*ej), w=exp(-0.8*ei),
and the exp(ei[n]) factor cancels in the softmax normalization, each
device only does ~1.5 elementwise passes over its [8192, 1024] score
block (computed transposed so PE aggregates without any transposes; a
ones column in the lhsT yields the softmax denominator for free).

Sharding: rows (target nodes) split across 8 cores; every core computes
h for all nodes from x (cheap) instead of an all-gather.

Unit types (per m-tile x head), engine-balanced via G_NUM/G_DEN:
 - path A: a = max(q2*w, q1) (DVE tensor_scalar)
 - G-split: max(q1,q2w)*adj = q1*adj + relu(q2w-q1)*adj. The q1 term
   needs NO elementwise pass: matmul lhsT=[h*q1|q1] against raw adj.
   Pre-op r = relu(q2*w - q1) on ACT.
Both paths then share ONE batched DVE mask-multiply per head-pair with
adj broadcast along the head dim via a stride-0 AP.
(scalar_tensor_tensor is avoided entirely: it runs in 1x DVE mode on
this silicon, slower than tensor_scalar+tensor_tensor.)

Phase A is pipelined with phase B (no barrier): w_bc first, then per
8-tile slab [matmul -> strided h/ej ACT copies -> slab exp/neg/hq1],
so phase-B units start while phase A is still producing later tiles.
"""

import sys

sys.path.insert(0, "/opt/trn_rl_repo")

import numpy as np
import ml_dtypes

import concourse.bass as bass
import concourse.mybir as mybir
from concourse import tile
from concourse.bass_utils import run_bass_kernel_spmd

F32 = mybir.dt.float32
BF16 = mybir.dt.bfloat16
AF = mybir.ActivationFunctionType
OP = mybir.AluOpType

N, IN, OUT, H = 8192, 128, 64, 4
NCORES = 8
EPS = 1e-5

# Fraction of (m-tile, head) units on the G-split path (ACT relu +
# extra PE stream) vs path A (DVE tensor_scalar). Balances DVE/ACT/PE.
G_NUM, G_DEN = 5, 9

# every GPS_TT_EVERY-th head-pair mask-multiply runs on GpSimd (else DVE)
GPS_TT_EVERY = 10**9

# m-tiles fetched per adjacency DMA (fewer, larger transfers)
DMA_BATCH = 4
# m-tiles per phase-A slab (q1/q2 exp granularity for pipelining)
SLAB = 8


def legalize_waits(nc, max_waits=1):
    """Walrus in this container encodes at most one inline sem-wait per
    engine instruction; hoist extras onto single-wait NoOps placed before."""
    nid = 0
    for f in nc.m.functions:
        for bb in f.blocks:
            new = []
            for inst in bb.instructions:
                si = inst.sync_info
                if si is not None and si.on_wait and len(si.on_wait) > max_waits:
                    waits = list(si.on_wait)
                    head, tail = waits[:-max_waits], waits[-max_waits:]
                    for w in head:
                        nid += 1
                        new.append(mybir.InstNoOp(
                            name=f"LGW-{nid}", ins=[], outs=[],
                            engine=inst.engine,
                            sync_info=mybir.SyncInfo(on_wait=[w], on_update=[]),
                            bass_nofuse=True,
                        ))
                    inst.sync_info = mybir.SyncInfo(
                        on_wait=tail, on_update=list(si.on_update)
                    )
                new.append(inst)
            bb.instructions = new
    return nc


def build_kernel(n_nodes=N, n_cores=NCORES, reps=1):
    """Build the per-core Bass program (SPMD: same program, per-core inputs)."""
    nblk = n_nodes // n_cores          # rows (target nodes) per core
    n_mt = n_nodes // 128              # m-tiles (source-node tiles of 128)
    jc = min(512, nblk)                # column-chunk width for PE moving dim
    n_jt = nblk // jc                  # column chunks of the n block
    WAUG = H * 65                      # per head: 64 h cols + ej
    dmab = min(DMA_BATCH, n_mt)        # m-tiles per adjacency DMA
    n_db = n_mt // dmab
    slab = min(SLAB, n_mt)
    n_sl = n_mt // slab

    # constants split in two: a small f32 tensor (w_bc inputs, DMA'd first
    # so phase A.0 starts early) and a bf16 tensor (xT + waug for phase A.1;
    # bf16 keeps the big DMA small and the phase-A matmul at full PE rate)
    CWW = nblk + H + 128
    CWX = n_nodes + WAUG
    nc = bass.Bass()
    constw_d = nc.dram_tensor("constw", [IN, CWW], F32, kind="ExternalInput")
    constx_d = nc.dram_tensor("constx", [IN, CWX], BF16, kind="ExternalInput")
    adjT_d = nc.dram_tensor("adjT", [n_nodes, nblk], BF16, kind="ExternalInput")
    out_d = nc.dram_tensor("out", [H, OUT + 1, nblk], F32, kind="ExternalOutput")

    with tile.TileContext(nc) as tc:
      for _rep in range(reps):
        with (
            tc.tile_pool(name="const", bufs=1) as cpool,
            tc.tile_pool(name="persist", bufs=1) as ppool,
            tc.tile_pool(name="stream", bufs=3) as spool,
            tc.tile_pool(name="score", bufs=4) as epool,
            tc.tile_pool(name="psA", bufs=2, space="PSUM") as psA_pool,
            tc.tile_pool(name="psW", bufs=1, space="PSUM") as psW_pool,
            tc.tile_pool(name="psB", bufs=1, space="PSUM") as psB_pool,
        ):
            # ---- load constants (small f32 first, then bf16 bulk) ----
            constw = cpool.tile([IN, CWW], F32, tag="constw")
            nc.sync.dma_start(constw[:], constw_d[:])
            constx = cpool.tile([IN, CWX], BF16, tag="constx")
            nc.sync.dma_start(constx[:], constx_d[:])
            xT = constx[:, 0:n_nodes]
            waug = constx[:, n_nodes:n_nodes + WAUG]
            xTown = constw[:, 0:nblk]
            wa = constw[:, nblk:nblk + H]
            ones_row = constw[0:1, CWW - 128:CWW]   # [1,128] of 1.0

            # ---- persistent state ----
            # h_all: per m-tile, 4 head blocks of [h(64) | 1]; only the ones
            # columns (col 64 of each 65-block) are memset (strided, cheap).
            h_all = ppool.tile([128, n_mt * WAUG], BF16, tag="hall", name="hall")
            # hq1_all: same layout, scaled by q1 -> [h*q1 | q1]
            hq1_all = ppool.tile([128, n_mt * WAUG], BF16, tag="hq1", name="hq1")
            ej_all = ppool.tile([128, H * n_mt], F32, tag="ejall", name="ejall")
            q1, q2, nq1, w_bc = [], [], [], []
            for hd in range(H):
                q1.append(ppool.tile([128, n_mt], F32, tag=f"q1_{hd}", name=f"q1_{hd}"))
                q2.append(ppool.tile([128, n_mt], F32, tag=f"q2_{hd}", name=f"q2_{hd}"))
                nq1.append(ppool.tile([128, n_mt], F32, tag=f"nq1_{hd}", name=f"nq1_{hd}"))
                w_bc.append(ppool.tile([128, nblk], BF16, tag=f"wbc{hd}", name=f"wbc{hd}"))

            # views for strided copies
            h_view = h_all[:].rearrange("p (t h c) -> p t h c", t=n_mt, h=H)
            hq1_v = hq1_all[:].rearrange("p (t h c) -> p t h c", t=n_mt, h=H)
            ej_view = ej_all[:].rearrange("p (h t) -> p h t", h=H)

            # ones columns of h_all (col 64 of each 65-block), strided memset
            nc.gpsimd.memset(h_view[:, :, :, 64:65], 1.0)

            # ---- phase A.0: w_bc for all heads (only needs consts) ----
            # j-chunked so phase A's PSUM footprint stays at 4 banks total
            for hd in range(H):
                ei_row = spool.tile([1, nblk], F32, tag="eirow")
                for j in range(n_jt):
                    jsl = slice(j * jc, (j + 1) * jc)
                    # ei_row = wa_i[hd] @ xTown via PE (lhsT free dim = 1)
                    eiT = psW_pool.tile([1, jc], F32, tag="eiT")
                    nc.tensor.matmul(
                        eiT[:], wa[:, hd:hd + 1], xTown[:, jsl],
                        start=True, stop=True,
                    )
                    nc.scalar.activation(ei_row[:, jsl], eiT[:], AF.Copy)
                    # broadcast ei row to all partitions via PE (ones x row),
                    # then w = exp(-0.8*ei) on the PSUM->SBUF copy
                    psW = psW_pool.tile([128, jc], F32, tag="psW")
                    nc.tensor.matmul(
                        psW[:], ones_row[:, :], ei_row[0:1, jsl],
                        start=True, stop=True,
                    )
                    nc.scalar.activation(
                        w_bc[hd][:, jsl], psW[:], AF.Exp, scale=-0.8
                    )

            # ---- phase A.1 slab emitter: h, ej, q1/q2, hq1 per slab ----
            # ej fused into the h matmul: waug cols per head = [W | W@a_j]
            def emit_slab(s):
                for t in range(s * slab, (s + 1) * slab):
                    psA = psA_pool.tile([128, WAUG], F32, tag="psA")
                    nc.tensor.matmul(
                        psA[:], xT[:, t * 128:(t + 1) * 128], waug[:],
                        start=True, stop=True,
                    )
                    psA_v = psA[:].rearrange("p (h c) -> p h c", h=H)
                    # h block: [128, 4 heads, 64] -> strided into h_all
                    nc.scalar.activation(
                        h_view[:, t, :, 0:64], psA_v[:, :, 0:64], AF.Copy
                    )
                    # ej stash: [128, 4 heads, 1] -> column t of each head
                    # row; on DVE, which is idle during phase A's ramp
                    nc.vector.tensor_copy(
                        ej_view[:, :, t:t + 1], psA_v[:, :, 64:65]
                    )
                sl_t = slice(s * slab, (s + 1) * slab)
                for hd in range(H):
                    nc.scalar.activation(
                        q1[hd][:, sl_t], ej_view[:, hd, sl_t], AF.Exp
                    )
                    nc.scalar.activation(
                        q2[hd][:, sl_t], ej_view[:, hd, sl_t], AF.Exp, scale=0.2
                    )
                    nc.vector.tensor_scalar_mul(
                        nq1[hd][:, sl_t], q1[hd][:, sl_t], -1.0
                    )
                    # G-split stationary blocks [h*q1 | q1]: q1 broadcast
                    # along the 65-col block via a stride-0 AP; on the
                    # otherwise-idle GpSimd engine (SBUF-only op)
                    q1_bc = (q1[hd][:, sl_t].rearrange("p (t c) -> p t c", c=1)
                             .broadcast_to((128, slab, 65)))
                    nc.vector.tensor_tensor(
                        hq1_v[:, sl_t, hd, :], h_view[:, sl_t, hd, :],
                        q1_bc, OP.mult,
                    )

            # ---- phase B emitters: masked attention + aggregation.
            # Two head-pair passes so the accumulators take only 4 PSUM
            # banks, leaving 4 for phase A to pipeline underneath. The
            # adjacency is streamed from DRAM once per pass (DMA has
            # headroom). Per tile: per-head pre-op (ACT relu or DVE
            # tensor_scalar) into half of acc2, then ONE batched DVE
            # mask-multiply for the pair with adj broadcast along the
            # head dim via a stride-0 AP.
            def emit_pass_batch(hp, ps_out, tb):
                adjt = spool.tile([128, dmab * nblk], BF16, tag="adjt")
                nc.sync.dma_start(
                    adjt[:].rearrange("p (a c) -> p a c", a=dmab),
                    adjT_d[tb * dmab * 128:(tb + 1) * dmab * 128, :]
                    .rearrange("(a p) c -> p a c", p=128),
                )
                for ti in range(dmab):
                    t = tb * dmab + ti
                    adjv = adjt[:, ti * nblk:(ti + 1) * nblk]
                    acc2 = epool.tile([128, 2 * nblk], BF16, tag="r")
                    for hi in range(2):
                        hd = hp * 2 + hi
                        unit = t * H + hd
                        q1s = q1[hd][:, t:t + 1]
                        q2s = q2[hd][:, t:t + 1]
                        dst = acc2[:, hi * nblk:(hi + 1) * nblk]
                        if unit % G_DEN < G_NUM:
                            # G-split pre-op: r = relu(q2*w - q1) on ACT
                            nc.scalar.activation(
                                dst, w_bc[hd][:], AF.Relu,
                                bias=nq1[hd][:, t:t + 1], scale=q2s,
                            )
                        else:
                            # path A pre-op: a = max(q2*w, q1) on DVE
                            nc.vector.tensor_scalar(
                                dst, w_bc[hd][:], q2s, q1s, OP.mult, OP.max
                            )
                    E2 = epool.tile([128, 2 * nblk], BF16, tag="E")
                    adj_bc = (adjv.rearrange("p (h c) -> p h c", h=1)
                              .broadcast_to((128, 2, nblk)))
                    nc.vector.tensor_tensor(
                        E2[:].rearrange("p (h c) -> p h c", h=2),
                        acc2[:].rearrange("p (h c) -> p h c", h=2),
                        adj_bc, OP.mult,
                    )
                    for hi in range(2):
                        hd = hp * 2 + hi
                        unit = t * H + hd
                        c0 = t * WAUG + hd * 65
                        for j in range(n_jt):
                            sl = slice(j * jc, (j + 1) * jc)
                            esl = slice(hi * nblk + j * jc,
                                        hi * nblk + (j + 1) * jc)
                            if unit % G_DEN < G_NUM:
                                nc.tensor.matmul(
                                    ps_out[hi][:, sl],
                                    hq1_all[:, c0:c0 + 65],
                                    adjv[:, sl],
                                    start=(t == 0), stop=False,
                                )
                                nc.tensor.matmul(
                                    ps_out[hi][:, sl],
                                    h_all[:, c0:c0 + 65],
                                    E2[:, esl],
                                    start=False, stop=(t == n_mt - 1),
                                )
                            else:
                                nc.tensor.matmul(
                                    ps_out[hi][:, sl],
                                    h_all[:, c0:c0 + 65],
                                    E2[:, esl],
                                    start=(t == 0), stop=(t == n_mt - 1),
                                )

            def emit_pass_tail(hp, ps_out):
                # emit [h + rowsum] rows for this pass; releases the PSUM
                # slots for the next pass. normalization on host
                for hi in range(2):
                    hd = hp * 2 + hi
                    o = spool.tile([OUT + 1, nblk], F32, tag="onorm")
                    nc.scalar.activation(o[:], ps_out[hi][:], AF.Copy)
                    nc.sync.dma_start(out_d[hd], o[:])

            # ---- software pipeline: interleave phase-A slabs with pass-1
            # tile batches so engine FIFOs don't serialize the handoff.
            # Slab s is emitted before pass-1 tiles of slab s-2, keeping
            # phase A ~2 slabs (16 tiles) ahead of phase B. ----
            bat_per_slab = max(1, slab // dmab)
            ps_out0 = [
                psB_pool.tile([OUT + 1, nblk], F32, tag=f"psB{hi}",
                              name=f"psB0_{hi}")
                for hi in range(2)
            ]
            lead = min(2, n_sl)
            for s in range(lead):
                emit_slab(s)
            for s in range(lead, n_sl):
                emit_slab(s)
                for b in range(bat_per_slab):
                    emit_pass_batch(0, ps_out0, (s - lead) * bat_per_slab + b)
            for tb in range((n_sl - lead) * bat_per_slab, n_db):
                emit_pass_batch(0, ps_out0, tb)
            emit_pass_tail(0, ps_out0)

            ps_out1 = [
                psB_pool.tile([OUT + 1, nblk], F32, tag=f"psB{hi}",
                              name=f"psB1_{hi}")
                for hi in range(2)
            ]
            for tb in range(n_db):
                emit_pass_batch(1, ps_out1, tb)
            emit_pass_tail(1, ps_out1)

    return nc


_CACHE = {}


def _get_nc(n_nodes, n_cores):
    key = (n_nodes, n_cores)
    if key not in _CACHE:
        _CACHE[key] = legalize_waits(build_kernel(n_nodes, n_cores))
    return _CACHE[key]


def make_in_maps(x, adj, W, a_i, a_j, n_cores=NCORES):
    n_nodes = x.shape[0]
    nblk = n_nodes // n_cores
    xT = np.ascontiguousarray(x.T).astype(np.float32)
    adjT = np.ascontiguousarray(adj.T).astype(ml_dtypes.bfloat16)
    WAUGW = H * 65
    waug = np.zeros((IN, H, 65), np.float32)
    wa = np.zeros((IN, H), np.float32)
    for hd in range(H):
        waug[:, hd, 0:64] = W[hd]
        waug[:, hd, 64] = W[hd] @ a_j[hd]
        wa[:, hd] = W[hd] @ a_i[hd]
    waug = waug.reshape(IN, WAUGW)
    constx = np.concatenate([xT, waug], axis=1).astype(ml_dtypes.bfloat16)
    maps = []
    for c in range(n_cores):
        sl = slice(c * nblk, (c + 1) * nblk)
        ones = np.zeros((IN, 128), np.float32)
        ones[0, :] = 1.0
        constw = np.concatenate(
            [xT[:, sl], wa, ones], axis=1
        ).astype(np.float32)
        maps.append({
            "constw": np.ascontiguousarray(constw),
            "constx": np.ascontiguousarray(constx),
            "adjT": np.ascontiguousarray(adjT[:, sl]),
        })
    return maps


def postprocess(results, gamma, beta, n_cores=NCORES):
    """Per-core [H, 65, nblk] -> full [N, H*OUT] with softmax-norm + BN + ReLU."""
    blocks = []
    for c in range(n_cores):
        r = results[c]["out"]                      # [H, 65, nblk]
        o = r[:, :OUT, :] / r[:, OUT:OUT + 1, :]   # softmax normalize
        # [H, OUT, nblk] -> [nblk, H*OUT]
        blocks.append(np.transpose(o, (2, 0, 1)).reshape(-1, H * OUT))
    out = np.concatenate(blocks, axis=0).astype(np.float32)
    mean = out.mean(axis=0)
    var = out.var(axis=0)
    out = (out - mean) * (1.0 / np.sqrt(var + EPS)) * gamma + beta
    return np.maximum(out, 0.0).astype(np.float32)


def kernel(x, adj, W, a_i, a_j, gamma, beta):
    nc = _get_nc(N, NCORES)
    in_maps = make_in_maps(x, adj, W, a_i, a_j, NCORES)
    res = run_bass_kernel_spmd(nc, in_maps, list(range(NCORES)))
    return postprocess(res.results, np.asarray(gamma), np.asarray(beta), NCORES)
